# revision 14
# baseline (speedup 1.0000x reference)
"""JointNetwork Trainium2 kernel.

out[b,t,u,f] = (audio[b] @ W[:H])[t,f] + (label[b] @ W[H:])[u,f] + bias[f]

Sharding: data-parallel over B — B=8 batch elements map 1:1 onto the 8
NeuronCores; no communication.

Per-core plan (memory regime: the 64 MiB fp32 output write at the ~358 GB/s
per-core HBM cap dominates; measured ~205 us vs ~187 us write roofline):
  1. Inputs stream in as bf16 (halves load bytes).  PE transposes build the
     [H, T]/[H, U] stationary operands; bf16 matmuls compute a = audio@Wa
     [256,1024] and l = label@Wl + b [64,1024] (PSUM accumulates fp32).
  2. Streams 128 output tiles of [128 rows, 1024]: rows = 2 t-values x 64
     u-values.  PE broadcasts a-rows into PSUM with one-hot selection
     matmuls (bf16: 1 cyc/row + fast weight load); l_tiled [l; l] is
     materialized once in fp32.
  3. Two copy paths drain PSUM, balancing engines: most tiles take DVE
     tensor_add(psum, l_tiled) -> SBUF; every third tile keeps the l-add on
     PE (selL matmuls) with an ACT copy.  Out-DMAs split across both HWDGE
     rings (sync + scalar) and sustain ~350 GB/s to HBM.
"""

import numpy as np

B, T, U, H, F = 8, 256, 64, 512, 1024
N_CORES = 8
NTILES = (T * U) // 128  # 128 output tiles of [128, F] per core
TPC = T // 128  # t-chunks (a row chunks)
KC = H // 128  # contraction chunks for projections

# broadcast-stage matmul dtype: "f32r" (fast, fp32 bits single-pass),
# "f32" (exact, 4x slower), "bf16" (fast, rounds a/l to bf16)
BCAST = "bf16"
OUT_BUFS = 20
PSUM_BUFS = 4
ACT_EVERY = 3


def _build_nc():
    import concourse.bacc as bacc
    import concourse.mybir as mybir
    import concourse.tile as tile

    f32 = mybir.dt.float32
    f32r = mybir.dt.float32r
    bf16 = mybir.dt.bfloat16
    bdt = {"f32r": f32r, "f32": f32, "bf16": bf16}[BCAST]

    nc = bacc.Bacc("TRN2", target_bir_lowering=False, debug=False)

    audio_d = nc.dram_tensor("audio", [T, H], bf16, kind="ExternalInput")
    label_d = nc.dram_tensor("label", [U, H], bf16, kind="ExternalInput")
    w_d = nc.dram_tensor("w", [2 * H, F], bf16, kind="ExternalInput")
    bias_d = nc.dram_tensor("bias", [1, F], bf16, kind="ExternalInput")
    sell_d = nc.dram_tensor("sell", [U, 128], bdt, kind="ExternalInput")
    ident_d = nc.dram_tensor("ident", [128, 128], bf16, kind="ExternalInput")
    ones_d = nc.dram_tensor("ones", [1, U], bf16, kind="ExternalInput")
    out_d = nc.dram_tensor("out", [T * U, F], f32, kind="ExternalOutput")

    out_view = out_d.rearrange("(n p) f -> n p f", p=128)

    with tile.TileContext(nc) as tc:
        with (
            tc.tile_pool(name="const", bufs=1) as cpool,
            tc.tile_pool(name="w", bufs=1) as wpool,
            tc.tile_pool(name="proj", bufs=1) as ppool,
            tc.tile_pool(name="psum", bufs=PSUM_BUFS, space="PSUM") as ps_pool,
            tc.tile_pool(name="out", bufs=OUT_BUFS) as opool,
        ):
            # ---- load inputs ----
            ident = cpool.tile([128, 128], bf16)
            nc.scalar.dma_start(out=ident[:], in_=ident_d[:])
            ones = cpool.tile([1, U], bf16)
            nc.scalar.dma_start(out=ones[:], in_=ones_d[:])
            bias = cpool.tile([1, F], bf16)
            nc.scalar.dma_start(out=bias[:], in_=bias_d[:])
            # sela[k, j*128+m] = 1.0 where k == 2j + (m>=64): built on-device.
            # affine_select: out = (base + ch_mult*part + pattern.freeidx <cmp> 0) ? in_ : fill
            sela = cpool.tile([128, 64 * 128], bdt)
            nc.gpsimd.memset(sela[:], 0.0)
            sela3 = sela.rearrange("p (j m) -> p j m", m=128)
            for half, base in ((0, 0), (1, -1)):
                nc.gpsimd.affine_select(
                    out=sela3[:, :, half * 64 : (half + 1) * 64],
                    in_=sela3[:, :, half * 64 : (half + 1) * 64],
                    compare_op=mybir.AluOpType.not_equal,
                    fill=1.0,
                    base=base,
                    pattern=[[-2, 64], [0, 64]],
                    channel_multiplier=1,
                )
            sell = cpool.tile([U, 128], bdt)
            nc.scalar.dma_start(out=sell[:], in_=sell_d[:])

            # wl half (needed first, feeds l/lt) on the scalar ring; wa on sync
            wtiles = [None] * (2 * KC)
            for k in range(KC, 2 * KC):
                wt = wpool.tile([128, F], bf16, tag=f"w{k}", name=f"w{k}")
                nc.scalar.dma_start(out=wt[:], in_=w_d[k * 128 : (k + 1) * 128, :])
                wtiles[k] = wt
            for k in range(KC):
                wt = wpool.tile([128, F], bf16, tag=f"w{k}", name=f"w{k}")
                nc.sync.dma_start(out=wt[:], in_=w_d[k * 128 : (k + 1) * 128, :])
                wtiles[k] = wt

            audio_sb = []
            for c in range(TPC):
                at = ppool.tile([128, H], bf16, tag=f"audio{c}", name=f"audio{c}")
                nc.scalar.dma_start(out=at[:], in_=audio_d[c * 128 : (c + 1) * 128, :])
                audio_sb.append(at)
            label_sb = ppool.tile([U, H], bf16, tag="label")
            nc.scalar.dma_start(out=label_sb[:], in_=label_d[:])

            # ---- transposes: audioT[k] = audio[:, k*128:+128].T  [128, T] ----
            audio_t = [ppool.tile([128, T], bf16, tag=f"at{k}", name=f"at{k}") for k in range(KC)]
            label_t = [ppool.tile([128, U], bf16, tag=f"lt{k}", name=f"lt{k}") for k in range(KC)]
            for k in range(KC):
                pt = ps_pool.tile([128, 2 * F], bf16, tag="ps", name="pt")
                nc.tensor.transpose(
                    pt[:, 0:U], label_sb[:, k * 128 : (k + 1) * 128], ident[0:U, 0:U]
                )
                nc.scalar.copy(out=label_t[k][:], in_=pt[:, 0:U])
                for c in range(TPC):
                    pt = ps_pool.tile([128, 2 * F], bf16, tag="ps", name="pt")
                    nc.tensor.transpose(
                        pt[:, 0:128], audio_sb[c][:, k * 128 : (k + 1) * 128], ident[:]
                    )
                    nc.scalar.copy(
                        out=audio_t[k][:, c * 128 : (c + 1) * 128], in_=pt[:, 0:128]
                    )

            # ---- projections (fp32) ----
            l_sb = ppool.tile([U, F], bdt, tag="l")
            pl = ps_pool.tile([128, F], f32, tag="ps", name="pl")
            for nh in range(2):
                sl = slice(nh * 512, (nh + 1) * 512)
                for k in range(KC):
                    nc.tensor.matmul(
                        pl[0:U, sl],
                        lhsT=label_t[k][:, 0:U],
                        rhs=wtiles[KC + k][:, sl],
                        start=(k == 0),
                        stop=False,
                    )
                nc.tensor.matmul(
                    pl[0:U, sl],
                    lhsT=ones[:, 0:U],
                    rhs=bias[:, sl],
                    start=False,
                    stop=True,
                )
            nc.scalar.copy(out=l_sb[:], in_=pl[0:U, :])

            # l_tiled [128, F] = [l; l] (fp32, includes bias) for DVE adds
            lt_sb = ppool.tile([128, F], f32, tag="lt")
            plt = ps_pool.tile([128, F], f32, tag="ps", name="plt")
            for nh in range(2):
                sl = slice(nh * 512, (nh + 1) * 512)
                nc.tensor.matmul(
                    plt[:, sl], lhsT=sell[:, :], rhs=l_sb[:, sl], start=True, stop=True
                )
            nc.scalar.copy(out=lt_sb[:], in_=plt[:])

            a_sb = [ppool.tile([128, F], bdt, tag=f"a{c}", name=f"a{c}") for c in range(TPC)]
            for c in range(TPC):
                pa = ps_pool.tile([128, F], f32, tag="ps", name="pa")
                for nh in range(2):
                    sl = slice(nh * 512, (nh + 1) * 512)
                    for k in range(KC):
                        nc.tensor.matmul(
                            pa[:, sl],
                            lhsT=audio_t[k][:, c * 128 : (c + 1) * 128],
                            rhs=wtiles[k][:, sl],
                            start=(k == 0),
                            stop=(k == KC - 1),
                        )
                nc.scalar.copy(out=a_sb[c][:], in_=pa[:])


            # ---- broadcast-add stream ----
            for i in range(NTILES):
                c, j = divmod(i, 64)
                act_tile = i % ACT_EVERY == 0
                po = ps_pool.tile([128, F], f32, tag="ps", name="po")
                for nh in range(2):
                    sl = slice(nh * 512, (nh + 1) * 512)
                    nc.tensor.matmul(
                        po[:, sl],
                        lhsT=sela[:, j * 128 : (j + 1) * 128],
                        rhs=a_sb[c][:, sl],
                        start=True,
                        stop=not act_tile,
                    )
                ot = opool.tile([128, F], f32)
                if act_tile:
                    # PE adds l_tiled via selL matmuls; ACT copies out
                    for nh in range(2):
                        sl = slice(nh * 512, (nh + 1) * 512)
                        nc.tensor.matmul(
                            po[:, sl],
                            lhsT=sell[:, :],
                            rhs=l_sb[:, sl],
                            start=False,
                            stop=True,
                        )
                    nc.scalar.copy(out=ot[:], in_=po[:])
                    nc.scalar.dma_start(out=out_view[i], in_=ot[:])
                else:
                    # DVE adds l_tiled during the PSUM->SBUF move
                    nc.vector.tensor_add(out=ot[:], in0=po[:], in1=lt_sb[:])
                    nc.sync.dma_start(out=out_view[i], in_=ot[:])

    nc.compile()
    return nc


_NC = None


def _get_nc():
    global _NC
    if _NC is None:
        _NC = _build_nc()
    return _NC


def _host_consts():
    import ml_dtypes

    seldt = {"bf16": ml_dtypes.bfloat16, "f32r": np.float32, "f32": np.float32}[BCAST]
    sell = np.zeros((U, 128), dtype=seldt)
    for m in range(128):
        sell[m % U, m] = 1.0
    ident = np.eye(128, dtype=np.float32)
    ones = np.ones((1, U), dtype=np.float32)
    return sell, ident, ones


def _in_maps(audio_vector, label_vector, W, b):
    import ml_dtypes

    bf = ml_dtypes.bfloat16
    sell, ident, ones = _host_consts()
    wb = np.ascontiguousarray(W).astype(bf)
    maps = []
    for i in range(N_CORES):
        maps.append(
            {
                "audio": np.ascontiguousarray(audio_vector[i]).astype(bf),
                "label": np.ascontiguousarray(label_vector[i]).astype(bf),
                "w": wb,
                "bias": np.ascontiguousarray(b).astype(bf).reshape(1, F),
                "sell": sell,
                "ident": ident.astype(bf),
                "ones": ones.astype(bf),
            }
        )
    return maps


def _run(in_maps, **kw):
    from concourse.bass_utils import run_bass_kernel_spmd

    nc = _get_nc()
    return run_bass_kernel_spmd(nc, in_maps, core_ids=list(range(N_CORES)), **kw)


def kernel(audio_vector, label_vector, W, b):
    res = _run(_in_maps(audio_vector, label_vector, W, b))
    out = np.stack([res.results[i]["out"].reshape(T, U, F) for i in range(N_CORES)])
    return out


# revision 16
# speedup vs baseline: 1.0239x; 1.0239x over previous
"""JointNetwork Trainium2 kernel.

out[b,t,u,f] = (audio[b] @ W[:H])[t,f] + (label[b] @ W[H:])[u,f] + bias[f]

Sharding: data-parallel over B — B=8 batch elements map 1:1 onto the 8
NeuronCores; no communication.

Per-core plan (memory regime: the 64 MiB fp32 output write at the ~358 GB/s
per-core HBM cap dominates; measured ~205 us vs ~187 us write roofline):
  1. Inputs stream in as bf16 (halves load bytes).  PE transposes build the
     [H, T]/[H, U] stationary operands; bf16 matmuls compute a = audio@Wa
     [256,1024] and l = label@Wl + b [64,1024] (PSUM accumulates fp32).
  2. Streams 128 output tiles of [128 rows, 1024]: rows = 2 t-values x 64
     u-values.  PE broadcasts a-rows into PSUM with one-hot selection
     matmuls (bf16: 1 cyc/row + fast weight load); l_tiled [l; l] is
     materialized once in fp32.
  3. Two copy paths drain PSUM, balancing engines: most tiles take DVE
     tensor_add(psum, l_tiled) -> SBUF; every third tile keeps the l-add on
     PE (selL matmuls) with an ACT copy.  Out-DMAs split across both HWDGE
     rings (sync + scalar) and sustain ~350 GB/s to HBM.
"""

import numpy as np

B, T, U, H, F = 8, 256, 64, 512, 1024
N_CORES = 8
NTILES = (T * U) // 128  # 128 output tiles of [128, F] per core
TPC = T // 128  # t-chunks (a row chunks)
KC = H // 128  # contraction chunks for projections

# broadcast-stage matmul dtype: "f32r" (fast, fp32 bits single-pass),
# "f32" (exact, 4x slower), "bf16" (fast, rounds a/l to bf16)
BCAST = "bf16"
OUT_BUFS = 20
PSUM_BUFS = 4
ACT_EVERY = 4


def _build_nc():
    import concourse.bacc as bacc
    import concourse.mybir as mybir
    import concourse.tile as tile

    f32 = mybir.dt.float32
    f32r = mybir.dt.float32r
    bf16 = mybir.dt.bfloat16
    bdt = {"f32r": f32r, "f32": f32, "bf16": bf16}[BCAST]

    nc = bacc.Bacc("TRN2", target_bir_lowering=False, debug=False)

    audio_d = nc.dram_tensor("audio", [T, H], bf16, kind="ExternalInput")
    label_d = nc.dram_tensor("label", [U, H], bf16, kind="ExternalInput")
    w_d = nc.dram_tensor("w", [2 * H, F], bf16, kind="ExternalInput")
    bias_d = nc.dram_tensor("bias", [1, F], bf16, kind="ExternalInput")
    sela_d = nc.dram_tensor("sela", [128, 64 * 128], bdt, kind="ExternalInput")
    sell_d = nc.dram_tensor("sell", [U, 128], bdt, kind="ExternalInput")
    ident_d = nc.dram_tensor("ident", [128, 128], bf16, kind="ExternalInput")
    ones_d = nc.dram_tensor("ones", [1, U], bf16, kind="ExternalInput")
    out_d = nc.dram_tensor("out", [T * U, F], f32, kind="ExternalOutput")

    out_view = out_d.rearrange("(n p) f -> n p f", p=128)

    with tile.TileContext(nc) as tc:
        with (
            tc.tile_pool(name="const", bufs=1) as cpool,
            tc.tile_pool(name="w", bufs=1) as wpool,
            tc.tile_pool(name="proj", bufs=1) as ppool,
            tc.tile_pool(name="psum", bufs=PSUM_BUFS, space="PSUM") as ps_pool,
            tc.tile_pool(name="out", bufs=OUT_BUFS) as opool,
        ):
            # ---- load inputs ----
            ident = cpool.tile([128, 128], bf16)
            nc.scalar.dma_start(out=ident[:], in_=ident_d[:])
            ones = cpool.tile([1, U], bf16)
            nc.scalar.dma_start(out=ones[:], in_=ones_d[:])
            bias = cpool.tile([1, F], bf16)
            nc.scalar.dma_start(out=bias[:], in_=bias_d[:])
            sela = cpool.tile([128, 64 * 128], bdt)
            nc.gpsimd.dma_start(out=sela[:], in_=sela_d[:])
            sell = cpool.tile([U, 128], bdt)
            nc.scalar.dma_start(out=sell[:], in_=sell_d[:])

            # wl half (needed first, feeds l/lt) on the scalar ring; wa on sync
            wtiles = [None] * (2 * KC)
            for k in range(KC, 2 * KC):
                wt = wpool.tile([128, F], bf16, tag=f"w{k}", name=f"w{k}")
                nc.scalar.dma_start(out=wt[:], in_=w_d[k * 128 : (k + 1) * 128, :])
                wtiles[k] = wt
            for k in range(KC):
                wt = wpool.tile([128, F], bf16, tag=f"w{k}", name=f"w{k}")
                nc.sync.dma_start(out=wt[:], in_=w_d[k * 128 : (k + 1) * 128, :])
                wtiles[k] = wt

            audio_sb = []
            for c in range(TPC):
                at = ppool.tile([128, H], bf16, tag=f"audio{c}", name=f"audio{c}")
                nc.scalar.dma_start(out=at[:], in_=audio_d[c * 128 : (c + 1) * 128, :])
                audio_sb.append(at)
            label_sb = ppool.tile([U, H], bf16, tag="label")
            nc.scalar.dma_start(out=label_sb[:], in_=label_d[:])

            # ---- transposes: audioT[k] = audio[:, k*128:+128].T  [128, T] ----
            audio_t = [ppool.tile([128, T], bf16, tag=f"at{k}", name=f"at{k}") for k in range(KC)]
            label_t = [ppool.tile([128, U], bf16, tag=f"lt{k}", name=f"lt{k}") for k in range(KC)]
            for k in range(KC):
                pt = ps_pool.tile([128, 2 * F], bf16, tag="ps", name="pt")
                nc.tensor.transpose(
                    pt[:, 0:U], label_sb[:, k * 128 : (k + 1) * 128], ident[0:U, 0:U]
                )
                nc.scalar.copy(out=label_t[k][:], in_=pt[:, 0:U])
                for c in range(TPC):
                    pt = ps_pool.tile([128, 2 * F], bf16, tag="ps", name="pt")
                    nc.tensor.transpose(
                        pt[:, 0:128], audio_sb[c][:, k * 128 : (k + 1) * 128], ident[:]
                    )
                    nc.scalar.copy(
                        out=audio_t[k][:, c * 128 : (c + 1) * 128], in_=pt[:, 0:128]
                    )

            # ---- projections (fp32) ----
            l_sb = ppool.tile([U, F], bdt, tag="l")
            pl = ps_pool.tile([128, F], f32, tag="ps", name="pl")
            for nh in range(2):
                sl = slice(nh * 512, (nh + 1) * 512)
                for k in range(KC):
                    nc.tensor.matmul(
                        pl[0:U, sl],
                        lhsT=label_t[k][:, 0:U],
                        rhs=wtiles[KC + k][:, sl],
                        start=(k == 0),
                        stop=False,
                    )
                nc.tensor.matmul(
                    pl[0:U, sl],
                    lhsT=ones[:, 0:U],
                    rhs=bias[:, sl],
                    start=False,
                    stop=True,
                )
            nc.scalar.copy(out=l_sb[:], in_=pl[0:U, :])

            # l_tiled [128, F] = [l; l] (fp32, includes bias) for DVE adds
            lt_sb = ppool.tile([128, F], f32, tag="lt")
            plt = ps_pool.tile([128, F], f32, tag="ps", name="plt")
            for nh in range(2):
                sl = slice(nh * 512, (nh + 1) * 512)
                nc.tensor.matmul(
                    plt[:, sl], lhsT=sell[:, :], rhs=l_sb[:, sl], start=True, stop=True
                )
            nc.scalar.copy(out=lt_sb[:], in_=plt[:])

            a_sb = [ppool.tile([128, F], bdt, tag=f"a{c}", name=f"a{c}") for c in range(TPC)]
            for c in range(TPC):
                pa = ps_pool.tile([128, F], f32, tag="ps", name="pa")
                for nh in range(2):
                    sl = slice(nh * 512, (nh + 1) * 512)
                    for k in range(KC):
                        nc.tensor.matmul(
                            pa[:, sl],
                            lhsT=audio_t[k][:, c * 128 : (c + 1) * 128],
                            rhs=wtiles[k][:, sl],
                            start=(k == 0),
                            stop=(k == KC - 1),
                        )
                nc.scalar.copy(out=a_sb[c][:], in_=pa[:])


            # ---- broadcast-add stream ----
            for i in range(NTILES):
                c, j = divmod(i, 64)
                act_tile = i % ACT_EVERY == 0
                po = ps_pool.tile([128, F], f32, tag="ps", name="po")
                for nh in range(2):
                    sl = slice(nh * 512, (nh + 1) * 512)
                    nc.tensor.matmul(
                        po[:, sl],
                        lhsT=sela[:, j * 128 : (j + 1) * 128],
                        rhs=a_sb[c][:, sl],
                        start=True,
                        stop=not act_tile,
                    )
                ot = opool.tile([128, F], f32)
                if act_tile:
                    # PE adds l_tiled via selL matmuls; ACT copies out
                    for nh in range(2):
                        sl = slice(nh * 512, (nh + 1) * 512)
                        nc.tensor.matmul(
                            po[:, sl],
                            lhsT=sell[:, :],
                            rhs=l_sb[:, sl],
                            start=False,
                            stop=True,
                        )
                    nc.scalar.copy(out=ot[:], in_=po[:])
                    nc.scalar.dma_start(out=out_view[i], in_=ot[:])
                else:
                    # DVE adds l_tiled during the PSUM->SBUF move
                    nc.vector.tensor_add(out=ot[:], in0=po[:], in1=lt_sb[:])
                    nc.sync.dma_start(out=out_view[i], in_=ot[:])

    nc.compile()
    return nc


_NC = None


def _get_nc():
    global _NC
    if _NC is None:
        _NC = _build_nc()
    return _NC


def _host_consts():
    import ml_dtypes

    seldt = {"bf16": ml_dtypes.bfloat16, "f32r": np.float32, "f32": np.float32}[BCAST]
    sela = np.zeros((128, 64 * 128), dtype=seldt)
    for j in range(64):
        for m in range(128):
            sela[2 * j + (1 if m >= 64 else 0), j * 128 + m] = 1.0
    sell = np.zeros((U, 128), dtype=seldt)
    for m in range(128):
        sell[m % U, m] = 1.0
    ident = np.eye(128, dtype=np.float32)
    ones = np.ones((1, U), dtype=np.float32)
    return sela, sell, ident, ones


def _in_maps(audio_vector, label_vector, W, b):
    import ml_dtypes

    bf = ml_dtypes.bfloat16
    sela, sell, ident, ones = _host_consts()
    wb = np.ascontiguousarray(W).astype(bf)
    maps = []
    for i in range(N_CORES):
        maps.append(
            {
                "audio": np.ascontiguousarray(audio_vector[i]).astype(bf),
                "label": np.ascontiguousarray(label_vector[i]).astype(bf),
                "w": wb,
                "bias": np.ascontiguousarray(b).astype(bf).reshape(1, F),
                "sela": sela,
                "sell": sell,
                "ident": ident.astype(bf),
                "ones": ones.astype(bf),
            }
        )
    return maps


def _run(in_maps, **kw):
    from concourse.bass_utils import run_bass_kernel_spmd

    nc = _get_nc()
    return run_bass_kernel_spmd(nc, in_maps, core_ids=list(range(N_CORES)), **kw)


def kernel(audio_vector, label_vector, W, b):
    res = _run(_in_maps(audio_vector, label_vector, W, b))
    out = np.stack([res.results[i]["out"].reshape(T, U, F) for i in range(N_CORES)])
    return out
